# revision 45
# baseline (speedup 1.0000x reference)
"""Trainium2 Bass kernel for a dense transformer block (attention + FFN).

Sharding: data-parallel over (batch, sequence-parity). 8 cores = 4 batches x 2
parity groups. Core c handles batch b = c//2 and the 128-row blocks of parity
p = c%2 (blocks p, p+2, ..., p+14) as query rows; K/V are computed for the
full sequence of the batch on-core (no collectives).

Performance structure (~495us baseline -> ~370us):
- The attention inner loop runs scores ONE key-block-pair ahead of the
  probs*V accumulation (psc PSUM double-buffering), so the PE never sits
  a full Exp latency at head-pair boundaries. This was worth 34us: the
  p-state clock stays ramped, dropping the average matmul time itself
  from 239ns to 210ns.
- Q/K/V projections, attention-out projection, and FFN2 run as fp8(e4m3)
  DoubleRow matmuls; weights are pre-scaled host-side (x64 / x16) to clear
  the fp8 subnormal band and descaled at PSUM eviction.
- V/K/Q projections are issued inside the LN1 loop as each group of 4
  sequence blocks completes (V as a per-group burst - per-BLOCK issue
  stalls the in-order PE queue and is ~70us slower), so the attention loop
  is a pure scores->exp->AV pipeline. K/Q PSUM evictions run on the
  Activation engine (Identity with bias+scale), V on DVE: phase-1 is
  otherwise DVE-bound while Act idles.
- Attention-out normalization is inline and software-pipelined one
  head-pair behind: raw row-sums (ones columns in V8: col 64 for even
  heads, col 32 for odd heads whose v block sits at cols 64:128 so the AV
  output lands partition-aligned) -> SBUF copies -> PE broadcast into the
  GARBAGE partition halves of the po PSUM banks themselves -> wide DVE
  reciprocal -> fused (po * SO/SV) * (1/rowsum) fp8 eviction
  (scalar_tensor_tensor, single PSUM operand - two PSUM operands on one
  DVE op is rejected by the BIR verifier, and GPSIMD cannot touch PSUM).
  pso bufs=2 so the next head-pair's AV accumulation overlaps the drain.
  NOTE: DVE InstReciprocal costs ~5.3ns/free-col regardless of partition
  count or dtype - batch it wide, never per-row.
- FFN1 is split-precision: contraction cols 0:512 as fp8 DoubleRow
  (h2T8/w1a), cols 512:768 bf16 (h2T16/w1b), both SW-scaled into one PSUM.
  Measured rel_err 0.0194 (gate 2e-2); full-fp8 FFN1 measures 0.0212.
- Phase 3 order: proj qb0-3 (fused PSUM-descale + residual-add via
  scalar_tensor_tensor) -> LN2 stats g0 (batched 4-wide, gpsimd) -> proj
  qb4-7 -> LN2 normalize g0 (z on DVE, transposes on the PE via
  is_transpose matmuls) -> stats g1 -> FFN1 g0 -> normalize g1 -> FFN1 g1
  -> FFN2 (fused descale+residual, direct DMA out).
- The causal mask is additive (-30000) into the score PSUM via one
  [P,2,P]-moving identity-stationary matmul per diagonal slice.
- v-bias and proj-bias are folded into xq host-side; xq and wp are DMA'd
  during attention. Prefetching w1/w2 during attention REGRESSES (~12us):
  the FFN weight loads fit fine in the post-attention DMA window.
"""
import sys

sys.path.insert(0, '/opt/trn_rl_repo')

import numpy as np
import ml_dtypes

import bass_rust
import concourse.bass as bass
import concourse.tile as tile
from concourse import mybir

P = 128
T = 2048
TQ = 1024
C = 768
H = 12
D = 64
FF = 3072
EO = C // P          # 6
EP = EO // 2         # 3 contraction pairs
MB = FF // P         # 24
MP = MB // 2         # 12
NB = T // P          # 16
NQ = TQ // P         # 8
HP = H // 2          # 6

f32 = mybir.dt.float32
bf16 = mybir.dt.bfloat16
fp8 = mybir.dt.float8e4
AF = mybir.ActivationFunctionType
ALU = mybir.AluOpType
DR = mybir.MatmulPerfMode.DoubleRow

# weight pre-scales (host side); descale factors live at PSUM eviction
SW = 64.0            # wq/wk/wv/w1/w2 host scale
SV = 8.0             # V8 = SV * v
SO = 16.0            # outT8 = SO * attn_out (after normalization)
SU = 16.0            # uT8 = SU * relu(u)
SP_ = 16.0           # wp host scale
MASK_NEG = -30000.0


def split_multiwait_instructions(nc):
    """The installed walrus build rejects any instruction carrying more than
    one sync wait; hoist extra waits onto NoOps inserted before it on the
    same (serial) engine."""
    n_fixed = 0
    for f in nc.m.functions:
        for bb in f.blocks:
            insts = bb.instructions
            new_insts = []
            dirty = False
            for inst in insts:
                si = inst.sync_info
                waits = list(si.on_wait) if si and si.on_wait else []
                if len(waits) > 1:
                    for j, w in enumerate(waits[:-1]):
                        nop = bass_rust.InstNoOp(
                            name=f"{inst.name}_sw{j}", ins=[], outs=[]
                        )
                        nop.engine = inst.engine
                        nop.sync_info = bass_rust.SyncInfo(
                            on_wait=[w], on_update=[]
                        )
                        new_insts.append(nop)
                    si.on_wait = waits[-1:]
                    dirty = True
                    n_fixed += 1
                new_insts.append(inst)
            if dirty:
                bb.instructions = new_insts
    return n_fixed


def build_program(skip_b2=False):
    """Build the single SPMD program (identical on all 8 cores)."""
    nc = bass.Bass("TRN2", target_bir_lowering=False, debug=False,
                   num_devices=8)

    xq_d = nc.declare_dram_parameter("xq", [TQ, C], f32, isOutput=False)
    xf_d = nc.declare_dram_parameter("xf", [T, C], bf16, isOutput=False)
    wq_d = nc.declare_dram_parameter("wq", [C, C], fp8, isOutput=False)
    wk_d = nc.declare_dram_parameter("wk", [C, C], fp8, isOutput=False)
    wv_d = nc.declare_dram_parameter("wv", [C, C], fp8, isOutput=False)
    wp_d = nc.declare_dram_parameter("wp", [C, C], fp8, isOutput=False)
    w1a_d = nc.declare_dram_parameter("w1a", [512, FF], fp8, isOutput=False)
    w1b_d = nc.declare_dram_parameter("w1b", [256, FF], bf16, isOutput=False)
    w2_d = nc.declare_dram_parameter("w2", [FF, C], fp8, isOutput=False)
    bq_d = nc.declare_dram_parameter("bq", [P, HP], f32, isOutput=False)
    bk_d = nc.declare_dram_parameter("bk", [P, HP], f32, isOutput=False)
    b1_d = nc.declare_dram_parameter("b1", [P, MB], f32, isOutput=False)
    b2_d = nc.declare_dram_parameter("b2", [1, C], bf16, isOutput=False)
    mk_d = nc.declare_dram_parameter("mk", [P, 2, 2, P], bf16, isOutput=False)
    ey_d = nc.declare_dram_parameter("eye", [P, P], bf16, isOutput=False)
    s0_d = nc.declare_dram_parameter("s0", [P, 1], f32, isOutput=False)
    s1_d = nc.declare_dram_parameter("s1", [P, 1], f32, isOutput=False)
    out_d = nc.declare_dram_parameter("out", [TQ, C], f32, isOutput=True)

    with tile.TileContext(nc) as tc:
        # --- pool stack (released LIFO) -------------------------------
        pers = tc.alloc_tile_pool(name="pers", bufs=1)
        late = tc.alloc_tile_pool(name="late", bufs=1)
        xqp = tc.alloc_tile_pool(name="xqp", bufs=1)
        attnp = tc.alloc_tile_pool(name="attnp", bufs=1)  # KT, QT, V8

        ones1 = pers.tile([1, P], bf16)
        nc.vector.memset(ones1[:], 1.0)
        bqc = pers.tile([P, HP], f32)
        nc.sync.dma_start(bqc[:], bq_d[:])
        bkc = pers.tile([P, HP], f32)
        nc.sync.dma_start(bkc[:], bk_d[:])
        b1c = pers.tile([P, MB], f32)
        b2r = pers.tile([1, C], bf16)
        mask2 = pers.tile([P, 2, 2, P], bf16)
        eye = pers.tile([P, P], bf16)
        s0c = pers.tile([P, 1], f32)
        nc.sync.dma_start(s0c[:], s0_d[:])
        s1c = pers.tile([P, 1], f32)
        nc.sync.dma_start(s1c[:], s1_d[:])


        outT_g = [late.tile([P, EO, 512], fp8, tag=f"outT{g}",
                            name=f"outT{g}") for g in range(2)]

        wp_t = late.tile([P, EO, C], fp8)
        h2T8_g = [late.tile([P, 4, 512], fp8, tag=f"h2T8{g}",
                            name=f"h2T8{g}") for g in range(2)]
        h2T16_g = [late.tile([P, 2, 512], bf16, tag=f"h2T16{g}",
                             name=f"h2T16{g}") for g in range(2)]
        x2s = late.tile([P, NQ, C], bf16)

        # residual rows (with folded biases): DMA'd after the LN1 loop
        xq_s = xqp.tile([P, NQ, C], f32)
        xq_t = [xq_s[:, qb, :] for qb in range(NQ)]

        KT_h = [attnp.tile([P, T], fp8, tag=f"KT{h}", name=f"KT{h}")
                for h in range(HP)]
        QT_h = [attnp.tile([P, TQ], fp8, tag=f"QT{h}", name=f"QT{h}")
                for h in range(HP)]
        V8 = attnp.tile([P, NB, H, P], fp8)
        # Even heads: cols 0:64 = 8*v, col 64 = ones (row-sum trick), rest
        # garbage. Odd heads mirrored: col 32 = ones, cols 64:128 = 8*v, so
        # their AV output lands at PSUM partitions 64:128 and the fused
        # normalization multiply has partition-aligned operands (PSUM
        # reads must start at a quadrant boundary, hence ones at 32).
        v8p = V8[:].rearrange("p nb (h2 two) d -> p nb h2 two d", two=2)
        nc.vector.memset(v8p[:, :, :, 0, 64:65], 1.0)
        nc.vector.memset(v8p[:, :, :, 1, 32:33], 1.0)

        # ---------------- Phase 1: LN1 + V/K/Q projections ----------------
        w13 = tc.alloc_tile_pool(name="w13", bufs=1)
        pq_ps = tc.alloc_tile_pool(name="pq_ps", bufs=2, space="PSUM")
        lnp = tc.alloc_tile_pool(name="lnp", bufs=3)
        lns = tc.alloc_tile_pool(name="lns", bufs=4)
        pv_ps = tc.alloc_tile_pool(name="pv_ps", bufs=3, space="PSUM")
        warm1_ps = tc.alloc_tile_pool(name="warm1_ps", bufs=1, space="PSUM")

        # startup warm chain: keep the PE p-state clock ramped while the
        # input DMAs and the first LN1 chains run (matmuls on memset data)
        wsrc = pers.tile([1, 512], bf16)
        nc.vector.memset(wsrc[:], 1.0)
        warm1 = warm1_ps.tile([P, 512], f32, tag="warm1", name="warm1")
        for wi in range(40):
            nc.tensor.matmul(warm1[:], ones1[:], wsrc[:],
                             start=True, stop=True, skip_group_check=True)

        xg_t = [None] * 4

        def xg_dma(dst, g4):
            for hh in range(2):
                nc.sync.dma_start(
                    dst[:, 2 * hh:2 * hh + 2, :],
                    xf_d.ap()[g4 * 512 + hh * 256:
                              g4 * 512 + (hh + 1) * 256, :].rearrange(
                        "(i p) c -> p i c", p=P))

        xg_t[0] = lnp.tile([P, 4, C], bf16, tag="ln_xg", bufs=3, name="xg0")
        xg_dma(xg_t[0], 0)
        wv_t = w13.tile([P, EO, C], fp8)
        nc.sync.dma_start(wv_t[:], wv_d.ap().rearrange("(o p) f -> p o f", p=P))
        # constants not needed until attention / FFN: loaded after the
        # startup-critical x and wv transfers
        nc.sync.dma_start(mask2[:], mk_d[:])
        nc.sync.dma_start(eye[:], ey_d[:])
        nc.sync.dma_start(b1c[:], b1_d[:])
        nc.sync.dma_start(b2r[:], b2_d[:])
        wq_t = w13.tile([P, EO, C], fp8)
        nc.sync.dma_start(wq_t[:], wq_d.ap().rearrange("(o p) f -> p o f", p=P))
        wk_t = w13.tile([P, EO, C], fp8)
        nc.sync.dma_start(wk_t[:], wk_d.ap().rearrange("(o p) f -> p o f", p=P))
        hT_g = [w13.tile([P, EO, 512], fp8, tag=f"hT{g}", name=f"hT{g}")
                for g in range(4)]
        hqT_g = [w13.tile([P, EO, 512], fp8, tag=f"hqT{g}", name=f"hqT{g}")
                 for g in range(2)]

        tb_prev = [None]

        def _ln1_body(b, tb):
            # hqT parity-select per block pair
            if b % 2 == 1:
                i = b // 2
                t0 = lnp.tile([P, EO, P], bf16, tag="hq_t0", name=f"hqt0_{i}")
                nc.vector.tensor_scalar(t0[:], tb_prev[0][:], s0c[:], None,
                                        ALU.mult)
                t1 = lnp.tile([P, EO, P], bf16, tag="hq_t1", name=f"hqt1_{i}")
                nc.vector.tensor_scalar(t1[:], tb[:], s1c[:], None, ALU.mult)
                nc.vector.tensor_tensor(
                    hqT_g[i // 4][:, :, (i % 4) * P:(i % 4 + 1) * P],
                    t0[:], t1[:], ALU.add)
            tb_prev[0] = tb

        def v_proj(cb):
            for fo in range(2):
                pv = pv_ps.tile([P, 384], f32, tag="pv", name=f"pv_{cb}_{fo}")
                for ep in range(EP):
                    nc.tensor.matmul(
                        pv[:],
                        hT_g[cb // 4][:, 2 * ep:2 * ep + 2,
                                      (cb % 4) * P:(cb % 4 + 1) * P],
                        wv_t[:, 2 * ep:2 * ep + 2, fo * 384:(fo + 1) * 384],
                        start=(ep == 0), stop=(ep == EP - 1), perf_mode=DR)
                pvr = pv[:].rearrange("p (h2 two d) -> p h2 two d",
                                      h2=3, two=2)
                v8r = V8[:, cb, fo * 6:(fo + 1) * 6, :].rearrange(
                    "p (h2 two) d -> p h2 two d", two=2)
                nc.vector.tensor_scalar(v8r[:, :, 0, 0:64],
                                        pvr[:, :, 0, :],
                                        SV / SW, None, ALU.mult)
                nc.vector.tensor_scalar(v8r[:, :, 1, 64:128],
                                        pvr[:, :, 1, :],
                                        SV / SW, None, ALU.mult)

        for g4 in range(4):
            if xg_t[g4] is None:
                xg_t[g4] = lnp.tile([P, 4, C], bf16, tag="ln_xg", bufs=3,
                                    name=f"xg{g4}")
                xg_dma(xg_t[g4], g4)
            xg = xg_t[g4]
            s1g = lns.tile([P, 4], f32, tag="ln_s1g", name=f"s1g{g4}")
            for hh in range(2):
                nc.vector.tensor_reduce(s1g[:, 2 * hh:2 * hh + 2],
                                        xg[:, 2 * hh:2 * hh + 2, :],
                                        mybir.AxisListType.X, ALU.add)
            s2g = lns.tile([P, 4], f32, tag="ln_s2g", name=f"s2g{g4}")
            for i in range(4):
                sq = lnp.tile([P, C], bf16, tag="ln_sq", bufs=4, name=f"sq{g4}_{i}")
                nc.scalar.activation(sq[:], xg[:, i, :], AF.Square,
                                     accum_out=s2g[:, i:i + 1])
            mug = lns.tile([P, 4], f32, tag="ln_mug", name=f"mug{g4}")
            nc.gpsimd.tensor_scalar(mug[:], s1g[:], 1.0 / C, None, ALU.mult)
            mu2g = lns.tile([P, 4], f32, tag="ln_mu2g", name=f"mu2g{g4}")
            nc.gpsimd.tensor_tensor(mu2g[:], mug[:], mug[:], ALU.mult)
            veg = lns.tile([P, 4], f32, tag="ln_veg", name=f"veg{g4}")
            nc.gpsimd.tensor_scalar(veg[:], s2g[:], 1.0 / C, 1e-5,
                                    ALU.mult, ALU.add)
            nc.gpsimd.tensor_tensor(veg[:], veg[:], mu2g[:], ALU.subtract)
            sdg = lns.tile([P, 4], f32, tag="ln_sdg", name=f"sdg{g4}")
            nc.scalar.activation(sdg[:], veg[:], AF.Sqrt)
            rstdg = lns.tile([P, 4], f32, tag="ln_rstdg", name=f"rstdg{g4}")
            nc.vector.reciprocal(rstdg[:], sdg[:])
            nbg = lns.tile([P, 4], f32, tag="ln_nbg", name=f"nbg{g4}")
            nc.gpsimd.tensor_tensor(nbg[:], mug[:], rstdg[:], ALU.mult)
            nc.gpsimd.tensor_scalar(nbg[:], nbg[:], -1.0, None, ALU.mult)
            for i in range(4):
                b = g4 * 4 + i
                z = lnp.tile([P, C], bf16, tag="ln_z", bufs=4, name=f"z_f{b}")
                nc.vector.tensor_scalar(z[:], xg[:, i, :],
                                        rstdg[:, i:i + 1], nbg[:, i:i + 1],
                                        ALU.mult, ALU.add)
                tb = lnp.tile([P, EO, P], bf16, tag="ln_tb", bufs=4, name=f"tb_f{b}")
                nc.sync.dma_start_transpose(tb[:], z[:])
                if b % 2 == 0:
                    nc.scalar.activation(hT_g[b // 4][:, :, (b % 4) * P:
                                                      (b % 4 + 1) * P],
                                         tb[:], AF.Identity)
                else:
                    nc.vector.tensor_copy(
                        out=hT_g[b // 4][:, :, (b % 4) * P:(b % 4 + 1) * P],
                        in_=tb[:])
                _ln1_body(b, tb)
            # V projection for this group's 4 key blocks
            for cb in range(g4 * 4, g4 * 4 + 4):
                v_proj(cb)
            # K projection: this group's 512 key columns, all head pairs
            for hp in range(HP):
                pk = pq_ps.tile([P, 512], f32, tag="pqkt",
                                name=f"pk_{hp}_{g4}")
                for ep in range(EP):
                    nc.tensor.matmul(
                        pk[:], wk_t[:, 2 * ep:2 * ep + 2, hp * P:(hp + 1) * P],
                        hT_g[g4][:, 2 * ep:2 * ep + 2, :],
                        start=(ep == 0), stop=(ep == EP - 1), perf_mode=DR)
                nc.scalar.activation(KT_h[hp][:, g4 * 512:(g4 + 1) * 512],
                                     pk[:], AF.Identity,
                                     bias=bkc[:, hp:hp + 1], scale=1.0 / SW)
            # Q projection: ready after each odd group (hqT_g[qc] complete)
            if g4 % 2 == 1:
                qc = g4 // 2
                for hp in range(HP):
                    pq = pq_ps.tile([P, 512], f32, tag="pqkt",
                                    name=f"pq_{hp}_{qc}")
                    for ep in range(EP):
                        nc.tensor.matmul(
                            pq[:],
                            wq_t[:, 2 * ep:2 * ep + 2, hp * P:(hp + 1) * P],
                            hqT_g[qc][:, 2 * ep:2 * ep + 2, :],
                            start=(ep == 0), stop=(ep == EP - 1), perf_mode=DR)
                    nc.scalar.activation(
                        QT_h[hp][:, qc * 512:(qc + 1) * 512],
                        pq[:], AF.Identity,
                        bias=bqc[:, hp:hp + 1], scale=1.0 / SW)

        for _pool in (warm1_ps, pv_ps, lns, lnp, pq_ps):
            _pool.release()

        # prefetch the projection weight + residual rows while attention runs
        nc.sync.dma_start(wp_t[:], wp_d.ap().rearrange("(o p) f -> p o f", p=P))
        nc.sync.dma_start(xq_s[:],
                          xq_d.ap().rearrange("(q p) c -> p q c", p=P))

        # ---------------- Phase 2: attention ------------------------------
        pr = tc.alloc_tile_pool(name="pr", bufs=3)
        rsn = tc.alloc_tile_pool(name="rsn", bufs=3)
        psc_ps = tc.alloc_tile_pool(name="psc", bufs=2, space="PSUM")
        pso_ps = tc.alloc_tile_pool(name="pso", bufs=2, space="PSUM")

        pending = [None]

        def emit_norm():
            # normalization of the PREVIOUS (hp, qc): raw row-sums (poA row
            # 64, poB row 32) -> SBUF, PE-broadcast into the GARBAGE
            # partition regions of the po banks themselves, wide-reciprocal
            # into SBUF, fp8 eviction fuses (po * SO/SV) * (1/rowsum).
            if pending[0] is None:
                return
            poA, poB, hp, qc = pending[0]
            pending[0] = None
            po_t = [poA, poB]
            c0 = rsn.tile([1, 512], bf16, tag="c0", name=f"c0_{hp}_{qc}")
            nc.vector.tensor_copy(out=c0[:], in_=poA[64:65, :])
            c1 = rsn.tile([1, 512], bf16, tag="c1", name=f"c1_{hp}_{qc}")
            nc.vector.tensor_copy(out=c1[:], in_=poB[32:33, :])
            nc.tensor.matmul(poB[0:64, :], ones1[:, 0:64], c0[:],
                             start=True, stop=True, skip_group_check=True)
            nc.tensor.matmul(poA[64:128, :], ones1[:, 0:64], c1[:],
                             start=True, stop=True, skip_group_check=True)
            pbs = rsn.tile([P, 512], bf16, tag="pbs", name=f"pbs_{hp}_{qc}")
            with nc.allow_low_precision(reason="1/rowsum as bf16 multiply "
                                        "operand"):
                nc.vector.reciprocal(pbs[0:64, :], poB[0:64, :])
                nc.vector.reciprocal(pbs[64:128, :], poA[64:128, :])
            for ab in range(2):
                lo = 64 * ab
                nc.vector.scalar_tensor_tensor(
                    outT_g[qc][lo:lo + 64, hp, :],
                    po_t[ab][lo:lo + 64, :], SO / SV,
                    pbs[lo:lo + 64, :], ALU.mult, ALU.mult)

        for hp in range(HP):
            for qc in range(2):
                q0 = qc * 512
                poA = pso_ps.tile([P, 512], f32, tag="poA", name=f"poA_{hp}_{qc}")
                poB = pso_ps.tile([P, 512], f32, tag="poB", name=f"poB_{hp}_{qc}")
                po_t = [poA, poB]
                npairs = 4 if qc == 0 else 8
                prb_h = {}

                def emit_scores(jp):
                    qsj = jp * P
                    qs = max(qsj, q0)
                    off = qs - q0
                    N = 512 - off
                    diag = (qs == qsj)
                    prb = pr.tile([P, 2, 2, 512], fp8, tag="prb", bufs=6,
                                  name=f"prb_{hp}_{qc}_{jp}")
                    prb_h[jp] = (prb, off, N)
                    for sl in range(2):
                        j = 2 * jp + sl
                        psc = psc_ps.tile([P, 2, 512], f32, tag="psc",
                                          name=f"psc_{hp}_{qc}_{j}")
                        for ab in range(2):
                            nc.tensor.matmul(
                                psc[:, ab, off:off + N],
                                KT_h[hp][64 * ab:64 * (ab + 1),
                                         j * P:(j + 1) * P],
                                QT_h[hp][64 * ab:64 * (ab + 1), qs:qs + N],
                                start=True, stop=not diag,
                                tile_position=(64 * ab, 0),
                                skip_group_check=True)
                        if diag:
                            nc.tensor.matmul(
                                psc[:, :, off:off + P],
                                eye[:], mask2[:, j % 2, :, :],
                                start=False, stop=True,
                                skip_group_check=True)
                        nc.scalar.activation(prb[:, sl, :, off:off + N],
                                             psc[:, :, off:off + N],
                                             AF.Exp, scale=0.125)

                # scores run one jp ahead of AV so the PE never waits a
                # full exp latency at the head-pair boundary
                emit_scores(0)
                for jp in range(npairs):
                    if jp + 1 < npairs:
                        emit_scores(jp + 1)
                    if jp == 0:
                        emit_norm()
                    prb, off, N = prb_h.pop(jp)
                    for ab in range(2):
                        nc.tensor.matmul(
                            po_t[ab][:, off:off + N],
                            V8[:, 2 * jp:2 * jp + 2, 2 * hp + ab, :],
                            prb[:, :, ab, off:off + N],
                            start=(jp == 0), stop=(jp == npairs - 1),
                            perf_mode=DR)
                pending[0] = (poA, poB, hp, qc)
        emit_norm()

        for _pool in (pso_ps, psc_ps, rsn, pr):
            _pool.release()
        w13.release()
        attnp.release()

        # -------- Phase 3: projection + residual + LN2 --------------------
        ffn = tc.alloc_tile_pool(name="ffn", bufs=1)
        w18_t = ffn.tile([P, 4, FF], fp8)
        nc.sync.dma_start(w18_t[:],
                          w1a_d.ap().rearrange("(o p) f -> p o f", p=P))
        w116_t = ffn.tile([P, 2, FF], bf16)
        nc.sync.dma_start(w116_t[:],
                          w1b_d.ap().rearrange("(o p) f -> p o f", p=P))
        w2_t = ffn.tile([P, MB, C], fp8)
        nc.sync.dma_start(w2_t[:],
                          w2_d.ap().rearrange("(m p) f -> p m f", p=P))
        uT_g = [ffn.tile([P, MB, 512], fp8, tag=f"uT{g}", name=f"uT{g}")
                for g in range(2)]

        lnp2 = tc.alloc_tile_pool(name="lnp2", bufs=4)
        lns2 = tc.alloc_tile_pool(name="lns2", bufs=4)
        ppr_ps = tc.alloc_tile_pool(name="ppr", bufs=2, space="PSUM")
        tr2_ps = tc.alloc_tile_pool(name="tr2_ps", bufs=2, space="PSUM")
        pu_ps = tc.alloc_tile_pool(name="pu", bufs=2, space="PSUM")

        # projection + residual: the PSUM descale and the residual add
        # fuse into one DVE scalar_tensor_tensor per half-block
        def proj_block(qb):
            for fo in range(2):
                pp = ppr_ps.tile([P, 384], f32, tag="ppr", name=f"pp_{qb}_{fo}")
                for fp in range(EP):
                    nc.tensor.matmul(
                        pp[:],
                        outT_g[qb // 4][:, 2 * fp:2 * fp + 2,
                                        (qb % 4) * P:(qb % 4 + 1) * P],
                        wp_t[:, 2 * fp:2 * fp + 2, fo * 384:(fo + 1) * 384],
                        start=(fp == 0), stop=(fp == EP - 1), perf_mode=DR)
                nc.vector.scalar_tensor_tensor(
                    x2s[:, qb, fo * 384:(fo + 1) * 384],
                    pp[:], 1.0 / (SO * SP_),
                    xq_s[:, qb, fo * 384:(fo + 1) * 384],
                    ALU.mult, ALU.add)

        # batched LN2 per group of 4 blocks, FFN1 for a group issued
        # right after its LN2 so the PE fills while the other group's
        # LN2 stats run on Act/DVE/GpSimd
        def ffn1_group(qc2):
            for mb in range(MB):
                pu = pu_ps.tile([P, 512], f32, tag="pu",
                                name=f"pu_{mb}_{qc2}")
                for dp in range(2):
                    nc.tensor.matmul(
                        pu[:],
                        w18_t[:, 2 * dp:2 * dp + 2, mb * P:(mb + 1) * P],
                        h2T8_g[qc2][:, 2 * dp:2 * dp + 2, :],
                        start=(dp == 0), stop=False, perf_mode=DR)
                for eo in range(2):
                    nc.tensor.matmul(
                        pu[:], w116_t[:, eo, mb * P:(mb + 1) * P],
                        h2T16_g[qc2][:, eo, :],
                        start=False, stop=(eo == 1))
                nc.scalar.activation(uT_g[qc2][:, mb, :], pu[:], AF.Relu,
                                     bias=b1c[:, mb:mb + 1], scale=SU / SW)

        def ln2_stats(g):
            s1g = lns2.tile([P, 4], f32, tag="l2_s1g", name=f"l2s1g{g}")
            nc.vector.tensor_reduce(s1g[:], x2s[:, 4 * g:4 * g + 4, :],
                                    mybir.AxisListType.X, ALU.add)
            s2g = lns2.tile([P, 4], f32, tag="l2_s2g", name=f"l2s2g{g}")
            for i in range(4):
                sq = lnp2.tile([P, C], bf16, tag="l2_sq", name=f"l2sq{g}_{i}")
                nc.scalar.activation(sq[:], x2s[:, 4 * g + i, :], AF.Square,
                                     accum_out=s2g[:, i:i + 1])
            mug = lns2.tile([P, 4], f32, tag="l2_mug", name=f"l2mug{g}")
            nc.gpsimd.tensor_scalar(mug[:], s1g[:], 1.0 / C, None, ALU.mult)
            mu2g = lns2.tile([P, 4], f32, tag="l2_mu2g", name=f"l2mu2g{g}")
            nc.gpsimd.tensor_tensor(mu2g[:], mug[:], mug[:], ALU.mult)
            veg = lns2.tile([P, 4], f32, tag="l2_veg", name=f"l2veg{g}")
            nc.gpsimd.tensor_scalar(veg[:], s2g[:], 1.0 / C, 1e-5,
                                    ALU.mult, ALU.add)
            nc.gpsimd.tensor_tensor(veg[:], veg[:], mu2g[:], ALU.subtract)
            sdg = lns2.tile([P, 4], f32, tag="l2_sdg", name=f"l2sdg{g}")
            nc.scalar.activation(sdg[:], veg[:], AF.Sqrt)
            rstdg = lns2.tile([P, 4], f32, tag="l2_rstdg", name=f"l2rstdg{g}")
            nc.vector.reciprocal(rstdg[:], sdg[:])
            nbg = lns2.tile([P, 4], f32, tag="l2_nbg", name=f"l2nbg{g}")
            nc.gpsimd.tensor_tensor(nbg[:], mug[:], rstdg[:], ALU.mult)
            nc.gpsimd.tensor_scalar(nbg[:], nbg[:], -1.0, None, ALU.mult)
            return rstdg, nbg

        def ln2_normalize(g, rstdg, nbg):
            for i in range(4):
                qb = 4 * g + i
                z = lnp2.tile([P, C], bf16, tag="l2_z", name=f"l2z{qb}")
                nc.vector.tensor_scalar(z[:], x2s[:, qb, :],
                                        rstdg[:, i:i + 1], nbg[:, i:i + 1],
                                        ALU.mult, ALU.add)
                tb = tr2_ps.tile([P, C], bf16, tag="l2_tb", name=f"l2tb{qb}")
                for eo in range(EO):
                    nc.tensor.matmul(tb[:, eo * P:(eo + 1) * P],
                                     z[:, eo * P:(eo + 1) * P], eye[:],
                                     is_transpose=True,
                                     skip_group_check=True)
                nc.scalar.activation(h2T8_g[g][:, :, i * P:(i + 1) * P],
                                     tb[:, 0:512].rearrange(
                                         "p (e q) -> p e q", e=4),
                                     AF.Identity)
                nc.vector.tensor_copy(
                    out=h2T16_g[g][:, :, i * P:(i + 1) * P],
                    in_=tb[:, 512:768].rearrange("p (e q) -> p e q", e=2))

        for qb in range(4):
            proj_block(qb)
        st0 = ln2_stats(0)
        for qb in range(4, NQ):
            proj_block(qb)
        ln2_normalize(0, *st0)
        st1 = ln2_stats(1)
        ffn1_group(0)
        ln2_normalize(1, *st1)
        ffn1_group(1)

        # -------- Phase 5: FFN2 -------------------------------------------
        oup = tc.alloc_tile_pool(name="oup", bufs=4)
        py_ps = tc.alloc_tile_pool(name="py", bufs=2, space="PSUM")

        for qb in range(NQ):
            ot = oup.tile([P, C], f32, tag="ot", name=f"ot_{qb}")
            for fo in range(2):
                py = py_ps.tile([P, 384], f32, tag="py", name=f"py_{qb}_{fo}")
                for mp in range(MP):
                    nc.tensor.matmul(
                        py[:],
                        uT_g[qb // 4][:, 2 * mp:2 * mp + 2,
                                      (qb % 4) * P:(qb % 4 + 1) * P],
                        w2_t[:, 2 * mp:2 * mp + 2, fo * 384:(fo + 1) * 384],
                        start=(mp == 0),
                        stop=(skip_b2 and mp == MP - 1), perf_mode=DR)
                if not skip_b2:
                    nc.tensor.matmul(py[:], ones1[:],
                                     b2r[:, fo * 384:(fo + 1) * 384],
                                     start=False, stop=True)
                nc.vector.scalar_tensor_tensor(
                    ot[:, fo * 384:(fo + 1) * 384], py[:], 1.0 / (SU * SW),
                    x2s[:, qb, fo * 384:(fo + 1) * 384],
                    ALU.mult, ALU.add)
            nc.sync.dma_start(out_d.ap()[qb * P:(qb + 1) * P, :], ot[:])

        for _pool in (py_ps, oup, pu_ps, tr2_ps, ppr_ps, lns2,
                      lnp2, ffn, xqp, late, pers):
            _pool.release()

    return nc


def prepare_in_maps(inputs):
    """Build the 8 per-core input maps from the full problem inputs."""
    x = np.asarray(inputs["x"], np.float32)
    wq = np.asarray(inputs["wq"], np.float32)
    wk = np.asarray(inputs["wk"], np.float32)
    wv = np.asarray(inputs["wv"], np.float32)
    w_proj = np.asarray(inputs["w_proj"], np.float32)
    b_proj = np.asarray(inputs["b_proj"], np.float32)
    w1 = np.asarray(inputs["w1"], np.float32)
    b1 = np.asarray(inputs["b1"], np.float32)
    w2 = np.asarray(inputs["w2"], np.float32)
    b2 = np.asarray(inputs["b2"], np.float32)
    g1 = np.asarray(inputs["ln1_g"], np.float32)
    be1 = np.asarray(inputs["ln1_b"], np.float32)
    g2 = np.asarray(inputs["ln2_g"], np.float32)
    be2 = np.asarray(inputs["ln2_b"], np.float32)

    bf = ml_dtypes.bfloat16
    f8 = ml_dtypes.float8_e4m3

    def to8(a):
        return np.clip(a, -224.0, 224.0).astype(f8)

    wq_r = wq.transpose(1, 0, 2).reshape(C, C)       # [c, h*d]
    wk_r = wk.transpose(1, 0, 2).reshape(C, C)
    wv_r = wv.transpose(1, 0, 2).reshape(C, C)
    wq8 = to8(SW * g1[:, None] * wq_r)
    wk8 = to8(SW * g1[:, None] * wk_r)
    wv8 = to8(SW * g1[:, None] * wv_r)
    bq = (be1 @ wq_r).reshape(HP, P).T.copy().astype(np.float32)   # [128, hp]
    bk = (be1 @ wk_r).reshape(HP, P).T.copy().astype(np.float32)
    bv_vec = (be1 @ wv_r).astype(np.float32)        # v bias, concat-head
    w1_s = SW * g2[:, None] * w1
    w1a8 = to8(w1_s[:512])
    w1b16 = w1_s[512:].astype(bf)
    b1f = (SU * (b1 + be2 @ w1)).reshape(MB, P).T.copy().astype(np.float32)
    wp8 = to8(SP_ * w_proj)
    w2_16 = to8(SW * w2)
    b2r = (SU * SW * b2).reshape(1, C).astype(bf)
    # bv and b_proj pass through attention/proj as constant rows; fold into xq
    xq_bias = (b_proj + bv_vec @ w_proj).astype(np.float32)

    ci = np.arange(P)[:, None]
    qi = np.arange(P)[None, :]
    tri_add = np.where(ci <= qi, 0.0, MASK_NEG).astype(np.float32)
    zer = np.zeros((P, P), np.float32)
    allm = np.full((P, P), MASK_NEG, np.float32)
    # mask2[c, jm, ab, q]: additive mask for diagonal key-block parity jm
    m_par = []
    for p in range(2):
        if p == 0:
            m0, m1 = tri_add, allm
        else:
            m0, m1 = zer, tri_add
        mk = np.stack([np.stack([m0, m0], 0), np.stack([m1, m1], 0)], 0)
        # mk[jm, ab, c, q] -> [c, jm, ab, q]
        m_par.append(np.ascontiguousarray(mk.transpose(2, 0, 1, 3)).astype(bf))

    eye16 = np.eye(P, dtype=np.float32).astype(bf)

    in_maps = []
    for core in range(8):
        b, p = core // 2, core % 2
        xf = np.ascontiguousarray(x[b]).astype(bf)
        xq = np.ascontiguousarray(
            x[b].reshape(NB, P, C)[p::2].reshape(TQ, C)) + xq_bias
        s0 = np.full((P, 1), 1.0 - p, np.float32)
        s1 = np.full((P, 1), float(p), np.float32)
        in_maps.append({
            "xq": xq, "xf": xf,
            "wq": wq8, "wk": wk8, "wv": wv8, "wp": wp8,
            "w1a": w1a8, "w1b": w1b16, "w2": w2_16,
            "bq": bq, "bk": bk, "b1": b1f, "b2": b2r,
            "mk": m_par[p], "eye": eye16, "s0": s0, "s1": s1,
        })
    return in_maps


def assemble_output(results):
    """Reassemble the 8 per-core [1024, 768] outputs into [4, 2048, 768]."""
    out = np.empty((4, T, C), np.float32)
    for core in range(8):
        b, p = core // 2, core % 2
        blocks = results[core]["out"].reshape(NQ, P, C)
        ov = out[b].reshape(NB, P, C)
        ov[p::2] = blocks
    return out


_CACHED_NC = None


def kernel(**inputs) -> np.ndarray:
    global _CACHED_NC
    from concourse.bass_utils import run_bass_kernel_spmd

    skip_b2 = bool(np.all(np.asarray(inputs["b2"]) == 0.0))
    if _CACHED_NC is None or _CACHED_NC[1] != skip_b2:
        nc = build_program(skip_b2=skip_b2)
        split_multiwait_instructions(nc)
        _CACHED_NC = (nc, skip_b2)
    in_maps = prepare_in_maps(inputs)
    res = run_bass_kernel_spmd(_CACHED_NC[0], in_maps, list(range(8)))
    return assemble_output(res.results)


# revision 46
# speedup vs baseline: 1.1624x; 1.1624x over previous
"""Trainium2 Bass kernel for a dense transformer block (attention + FFN).

Sharding: data-parallel over (batch, sequence-parity). 8 cores = 4 batches x 2
parity groups. Core c handles batch b = c//2 and the 128-row blocks of parity
p = c%2 (blocks p, p+2, ..., p+14) as query rows; K/V are computed for the
full sequence of the batch on-core (no collectives).

Performance structure (~495us baseline -> ~370us):
- The attention inner loop runs scores ONE key-block-pair ahead of the
  probs*V accumulation (psc PSUM double-buffering), so the PE never sits
  a full Exp latency at head-pair boundaries. This was worth 34us: the
  p-state clock stays ramped, dropping the average matmul time itself
  from 239ns to 210ns.
- Q/K/V projections, attention-out projection, and FFN2 run as fp8(e4m3)
  DoubleRow matmuls; weights are pre-scaled host-side (x64 / x16) to clear
  the fp8 subnormal band and descaled at PSUM eviction.
- V/K/Q projections are issued inside the LN1 loop as each group of 4
  sequence blocks completes (V as a per-group burst - per-BLOCK issue
  stalls the in-order PE queue and is ~70us slower), so the attention loop
  is a pure scores->exp->AV pipeline. K/Q PSUM evictions run on the
  Activation engine (Identity with bias+scale), V on DVE: phase-1 is
  otherwise DVE-bound while Act idles.
- Attention-out normalization is inline and software-pipelined one
  head-pair behind: raw row-sums (ones columns in V8: col 64 for even
  heads, col 32 for odd heads whose v block sits at cols 64:128 so the AV
  output lands partition-aligned) -> SBUF copies -> PE broadcast into the
  GARBAGE partition halves of the po PSUM banks themselves -> wide DVE
  reciprocal -> fused (po * SO/SV) * (1/rowsum) fp8 eviction
  (scalar_tensor_tensor, single PSUM operand - two PSUM operands on one
  DVE op is rejected by the BIR verifier, and GPSIMD cannot touch PSUM).
  pso bufs=2 so the next head-pair's AV accumulation overlaps the drain.
  NOTE: DVE InstReciprocal costs ~5.3ns/free-col regardless of partition
  count or dtype - batch it wide, never per-row.
- FFN1 is split-precision: contraction cols 0:512 as fp8 DoubleRow
  (h2T8/w1a), cols 512:768 bf16 (h2T16/w1b), both SW-scaled into one PSUM.
  Measured rel_err 0.0194 (gate 2e-2); full-fp8 FFN1 measures 0.0212.
- Phase 3 order: proj qb0-3 (fused PSUM-descale + residual-add via
  scalar_tensor_tensor) -> LN2 stats g0 (batched 4-wide, gpsimd) -> proj
  qb4-7 -> LN2 normalize g0 (z on DVE, transposes on the PE via
  is_transpose matmuls) -> stats g1 -> FFN1 g0 -> normalize g1 -> FFN1 g1
  -> FFN2 (fused descale+residual, direct DMA out).
- The causal mask is additive (-30000) into the score PSUM via one
  [P,2,P]-moving identity-stationary matmul per diagonal slice.
- v-bias and proj-bias are folded into xq host-side; xq and wp are DMA'd
  during attention. Prefetching w1/w2 during attention REGRESSES (~12us):
  the FFN weight loads fit fine in the post-attention DMA window.
"""
import sys

sys.path.insert(0, '/opt/trn_rl_repo')

import numpy as np
import ml_dtypes

import bass_rust
import concourse.bass as bass
import concourse.tile as tile
from concourse import mybir

P = 128
T = 2048
TQ = 1024
C = 768
H = 12
D = 64
FF = 3072
EO = C // P          # 6
EP = EO // 2         # 3 contraction pairs
MB = FF // P         # 24
MP = MB // 2         # 12
NB = T // P          # 16
NQ = TQ // P         # 8
HP = H // 2          # 6

f32 = mybir.dt.float32
bf16 = mybir.dt.bfloat16
fp8 = mybir.dt.float8e4
AF = mybir.ActivationFunctionType
ALU = mybir.AluOpType
DR = mybir.MatmulPerfMode.DoubleRow

# weight pre-scales (host side); descale factors live at PSUM eviction
SW = 64.0            # wq/wk/wv/w1/w2 host scale
SV = 8.0             # V8 = SV * v
SO = 16.0            # outT8 = SO * attn_out (after normalization)
SU = 16.0            # uT8 = SU * relu(u)
SP_ = 16.0           # wp host scale
MASK_NEG = -30000.0


def split_multiwait_instructions(nc):
    """The installed walrus build rejects any instruction carrying more than
    one sync wait; hoist extra waits onto NoOps inserted before it on the
    same (serial) engine."""
    n_fixed = 0
    for f in nc.m.functions:
        for bb in f.blocks:
            insts = bb.instructions
            new_insts = []
            dirty = False
            for inst in insts:
                si = inst.sync_info
                waits = list(si.on_wait) if si and si.on_wait else []
                if len(waits) > 1:
                    for j, w in enumerate(waits[:-1]):
                        nop = bass_rust.InstNoOp(
                            name=f"{inst.name}_sw{j}", ins=[], outs=[]
                        )
                        nop.engine = inst.engine
                        nop.sync_info = bass_rust.SyncInfo(
                            on_wait=[w], on_update=[]
                        )
                        new_insts.append(nop)
                    si.on_wait = waits[-1:]
                    dirty = True
                    n_fixed += 1
                new_insts.append(inst)
            if dirty:
                bb.instructions = new_insts
    return n_fixed


def build_program(skip_b2=False):
    """Build the single SPMD program (identical on all 8 cores)."""
    nc = bass.Bass("TRN2", target_bir_lowering=False, debug=False,
                   num_devices=8)

    xq_d = nc.declare_dram_parameter("xq", [TQ, C], f32, isOutput=False)
    xf_d = nc.declare_dram_parameter("xf", [T, C], bf16, isOutput=False)
    wq_d = nc.declare_dram_parameter("wq", [C, C], fp8, isOutput=False)
    wk_d = nc.declare_dram_parameter("wk", [C, C], fp8, isOutput=False)
    wv_d = nc.declare_dram_parameter("wv", [C, C], fp8, isOutput=False)
    wp_d = nc.declare_dram_parameter("wp", [C, C], fp8, isOutput=False)
    w1a_d = nc.declare_dram_parameter("w1a", [512, FF], fp8, isOutput=False)
    w1b_d = nc.declare_dram_parameter("w1b", [256, FF], bf16, isOutput=False)
    w2_d = nc.declare_dram_parameter("w2", [FF, C], fp8, isOutput=False)
    bq_d = nc.declare_dram_parameter("bq", [P, HP], f32, isOutput=False)
    bk_d = nc.declare_dram_parameter("bk", [P, HP], f32, isOutput=False)
    b1_d = nc.declare_dram_parameter("b1", [P, MB], f32, isOutput=False)
    b2_d = nc.declare_dram_parameter("b2", [1, C], bf16, isOutput=False)
    mk_d = nc.declare_dram_parameter("mk", [P, 2, 2, P], bf16, isOutput=False)
    ey_d = nc.declare_dram_parameter("eye", [P, P], bf16, isOutput=False)
    s0_d = nc.declare_dram_parameter("s0", [P, 1], f32, isOutput=False)
    s1_d = nc.declare_dram_parameter("s1", [P, 1], f32, isOutput=False)
    out_d = nc.declare_dram_parameter("out", [TQ, C], f32, isOutput=True)

    with tile.TileContext(nc) as tc:
        # --- pool stack (released LIFO) -------------------------------
        pers = tc.alloc_tile_pool(name="pers", bufs=1)
        late = tc.alloc_tile_pool(name="late", bufs=1)
        xqp = tc.alloc_tile_pool(name="xqp", bufs=1)
        attnp = tc.alloc_tile_pool(name="attnp", bufs=1)  # KT, QT, V8

        ones1 = pers.tile([1, P], bf16)
        nc.vector.memset(ones1[:], 1.0)
        bqc = pers.tile([P, HP], f32)
        nc.sync.dma_start(bqc[:], bq_d[:])
        bkc = pers.tile([P, HP], f32)
        nc.sync.dma_start(bkc[:], bk_d[:])
        b1c = pers.tile([P, MB], f32)
        nc.sync.dma_start(b1c[:], b1_d[:])
        b2r = pers.tile([1, C], bf16)
        nc.sync.dma_start(b2r[:], b2_d[:])
        mask2 = pers.tile([P, 2, 2, P], bf16)
        nc.sync.dma_start(mask2[:], mk_d[:])
        eye = pers.tile([P, P], bf16)
        nc.sync.dma_start(eye[:], ey_d[:])
        s0c = pers.tile([P, 1], f32)
        nc.sync.dma_start(s0c[:], s0_d[:])
        s1c = pers.tile([P, 1], f32)
        nc.sync.dma_start(s1c[:], s1_d[:])


        outT_g = [late.tile([P, EO, 512], fp8, tag=f"outT{g}",
                            name=f"outT{g}") for g in range(2)]

        wp_t = late.tile([P, EO, C], fp8)
        h2T8_g = [late.tile([P, 4, 512], fp8, tag=f"h2T8{g}",
                            name=f"h2T8{g}") for g in range(2)]
        h2T16_g = [late.tile([P, 2, 512], bf16, tag=f"h2T16{g}",
                             name=f"h2T16{g}") for g in range(2)]
        x2s = late.tile([P, NQ, C], bf16)

        # residual rows (with folded biases): DMA'd after the LN1 loop
        xq_s = xqp.tile([P, NQ, C], f32)
        xq_t = [xq_s[:, qb, :] for qb in range(NQ)]

        KT_h = [attnp.tile([P, T], fp8, tag=f"KT{h}", name=f"KT{h}")
                for h in range(HP)]
        QT_h = [attnp.tile([P, TQ], fp8, tag=f"QT{h}", name=f"QT{h}")
                for h in range(HP)]
        V8 = attnp.tile([P, NB, H, P], fp8)
        # Even heads: cols 0:64 = 8*v, col 64 = ones (row-sum trick), rest
        # garbage. Odd heads mirrored: col 32 = ones, cols 64:128 = 8*v, so
        # their AV output lands at PSUM partitions 64:128 and the fused
        # normalization multiply has partition-aligned operands (PSUM
        # reads must start at a quadrant boundary, hence ones at 32).
        v8p = V8[:].rearrange("p nb (h2 two) d -> p nb h2 two d", two=2)
        nc.vector.memset(v8p[:, :, :, 0, 64:65], 1.0)
        nc.vector.memset(v8p[:, :, :, 1, 32:33], 1.0)

        # ---------------- Phase 1: LN1 + V/K/Q projections ----------------
        w13 = tc.alloc_tile_pool(name="w13", bufs=1)
        pq_ps = tc.alloc_tile_pool(name="pq_ps", bufs=2, space="PSUM")
        lnp = tc.alloc_tile_pool(name="lnp", bufs=3)
        lns = tc.alloc_tile_pool(name="lns", bufs=4)
        pv_ps = tc.alloc_tile_pool(name="pv_ps", bufs=3, space="PSUM")
        warm1_ps = tc.alloc_tile_pool(name="warm1_ps", bufs=1, space="PSUM")

        # startup warm chain: keep the PE p-state clock ramped while the
        # input DMAs and the first LN1 chains run (matmuls on memset data)
        wsrc = pers.tile([1, 512], bf16)
        nc.vector.memset(wsrc[:], 1.0)
        warm1 = warm1_ps.tile([P, 512], f32, tag="warm1", name="warm1")
        for wi in range(40):
            nc.tensor.matmul(warm1[:], ones1[:], wsrc[:],
                             start=True, stop=True, skip_group_check=True)

        xg_t = [None] * 4

        def xg_dma(dst, g4):
            for hh in range(2):
                nc.sync.dma_start(
                    dst[:, 2 * hh:2 * hh + 2, :],
                    xf_d.ap()[g4 * 512 + hh * 256:
                              g4 * 512 + (hh + 1) * 256, :].rearrange(
                        "(i p) c -> p i c", p=P))

        xg_t[0] = lnp.tile([P, 4, C], bf16, tag="ln_xg", bufs=3, name="xg0")
        xg_dma(xg_t[0], 0)
        wv_t = w13.tile([P, EO, C], fp8)
        nc.sync.dma_start(wv_t[:], wv_d.ap().rearrange("(o p) f -> p o f", p=P))
        wq_t = w13.tile([P, EO, C], fp8)
        nc.sync.dma_start(wq_t[:], wq_d.ap().rearrange("(o p) f -> p o f", p=P))
        wk_t = w13.tile([P, EO, C], fp8)
        nc.sync.dma_start(wk_t[:], wk_d.ap().rearrange("(o p) f -> p o f", p=P))
        hT_g = [w13.tile([P, EO, 512], fp8, tag=f"hT{g}", name=f"hT{g}")
                for g in range(4)]
        hqT_g = [w13.tile([P, EO, 512], fp8, tag=f"hqT{g}", name=f"hqT{g}")
                 for g in range(2)]

        tb_prev = [None]

        def _ln1_body(b, tb):
            # hqT parity-select per block pair
            if b % 2 == 1:
                i = b // 2
                t0 = lnp.tile([P, EO, P], bf16, tag="hq_t0", name=f"hqt0_{i}")
                nc.vector.tensor_scalar(t0[:], tb_prev[0][:], s0c[:], None,
                                        ALU.mult)
                t1 = lnp.tile([P, EO, P], bf16, tag="hq_t1", name=f"hqt1_{i}")
                nc.vector.tensor_scalar(t1[:], tb[:], s1c[:], None, ALU.mult)
                nc.vector.tensor_tensor(
                    hqT_g[i // 4][:, :, (i % 4) * P:(i % 4 + 1) * P],
                    t0[:], t1[:], ALU.add)
            tb_prev[0] = tb

        def v_proj(cb):
            for fo in range(2):
                pv = pv_ps.tile([P, 384], f32, tag="pv", name=f"pv_{cb}_{fo}")
                for ep in range(EP):
                    nc.tensor.matmul(
                        pv[:],
                        hT_g[cb // 4][:, 2 * ep:2 * ep + 2,
                                      (cb % 4) * P:(cb % 4 + 1) * P],
                        wv_t[:, 2 * ep:2 * ep + 2, fo * 384:(fo + 1) * 384],
                        start=(ep == 0), stop=(ep == EP - 1), perf_mode=DR)
                pvr = pv[:].rearrange("p (h2 two d) -> p h2 two d",
                                      h2=3, two=2)
                v8r = V8[:, cb, fo * 6:(fo + 1) * 6, :].rearrange(
                    "p (h2 two) d -> p h2 two d", two=2)
                nc.vector.tensor_scalar(v8r[:, :, 0, 0:64],
                                        pvr[:, :, 0, :],
                                        SV / SW, None, ALU.mult)
                nc.vector.tensor_scalar(v8r[:, :, 1, 64:128],
                                        pvr[:, :, 1, :],
                                        SV / SW, None, ALU.mult)

        for g4 in range(4):
            if xg_t[g4] is None:
                xg_t[g4] = lnp.tile([P, 4, C], bf16, tag="ln_xg", bufs=3,
                                    name=f"xg{g4}")
                xg_dma(xg_t[g4], g4)
            xg = xg_t[g4]
            s1g = lns.tile([P, 4], f32, tag="ln_s1g", name=f"s1g{g4}")
            for hh in range(2):
                nc.vector.tensor_reduce(s1g[:, 2 * hh:2 * hh + 2],
                                        xg[:, 2 * hh:2 * hh + 2, :],
                                        mybir.AxisListType.X, ALU.add)
            s2g = lns.tile([P, 4], f32, tag="ln_s2g", name=f"s2g{g4}")
            for i in range(4):
                sq = lnp.tile([P, C], bf16, tag="ln_sq", bufs=4, name=f"sq{g4}_{i}")
                nc.scalar.activation(sq[:], xg[:, i, :], AF.Square,
                                     accum_out=s2g[:, i:i + 1])
            mug = lns.tile([P, 4], f32, tag="ln_mug", name=f"mug{g4}")
            nc.gpsimd.tensor_scalar(mug[:], s1g[:], 1.0 / C, None, ALU.mult)
            mu2g = lns.tile([P, 4], f32, tag="ln_mu2g", name=f"mu2g{g4}")
            nc.gpsimd.tensor_tensor(mu2g[:], mug[:], mug[:], ALU.mult)
            veg = lns.tile([P, 4], f32, tag="ln_veg", name=f"veg{g4}")
            nc.gpsimd.tensor_scalar(veg[:], s2g[:], 1.0 / C, 1e-5,
                                    ALU.mult, ALU.add)
            nc.gpsimd.tensor_tensor(veg[:], veg[:], mu2g[:], ALU.subtract)
            sdg = lns.tile([P, 4], f32, tag="ln_sdg", name=f"sdg{g4}")
            nc.scalar.activation(sdg[:], veg[:], AF.Sqrt)
            rstdg = lns.tile([P, 4], f32, tag="ln_rstdg", name=f"rstdg{g4}")
            nc.vector.reciprocal(rstdg[:], sdg[:])
            nbg = lns.tile([P, 4], f32, tag="ln_nbg", name=f"nbg{g4}")
            nc.gpsimd.tensor_tensor(nbg[:], mug[:], rstdg[:], ALU.mult)
            nc.gpsimd.tensor_scalar(nbg[:], nbg[:], -1.0, None, ALU.mult)
            for i in range(4):
                b = g4 * 4 + i
                z = lnp.tile([P, C], bf16, tag="ln_z", bufs=4, name=f"z_f{b}")
                nc.vector.tensor_scalar(z[:], xg[:, i, :],
                                        rstdg[:, i:i + 1], nbg[:, i:i + 1],
                                        ALU.mult, ALU.add)
                tb = lnp.tile([P, EO, P], bf16, tag="ln_tb", bufs=4, name=f"tb_f{b}")
                nc.sync.dma_start_transpose(tb[:], z[:])
                if b % 2 == 0:
                    nc.scalar.activation(hT_g[b // 4][:, :, (b % 4) * P:
                                                      (b % 4 + 1) * P],
                                         tb[:], AF.Identity)
                else:
                    nc.vector.tensor_copy(
                        out=hT_g[b // 4][:, :, (b % 4) * P:(b % 4 + 1) * P],
                        in_=tb[:])
                _ln1_body(b, tb)
            # V projection for this group's 4 key blocks
            for cb in range(g4 * 4, g4 * 4 + 4):
                v_proj(cb)
            # K projection: this group's 512 key columns, all head pairs
            for hp in range(HP):
                pk = pq_ps.tile([P, 512], f32, tag="pqkt",
                                name=f"pk_{hp}_{g4}")
                for ep in range(EP):
                    nc.tensor.matmul(
                        pk[:], wk_t[:, 2 * ep:2 * ep + 2, hp * P:(hp + 1) * P],
                        hT_g[g4][:, 2 * ep:2 * ep + 2, :],
                        start=(ep == 0), stop=(ep == EP - 1), perf_mode=DR)
                nc.scalar.activation(KT_h[hp][:, g4 * 512:(g4 + 1) * 512],
                                     pk[:], AF.Identity,
                                     bias=bkc[:, hp:hp + 1], scale=1.0 / SW)
            # Q projection: ready after each odd group (hqT_g[qc] complete)
            if g4 % 2 == 1:
                qc = g4 // 2
                for hp in range(HP):
                    pq = pq_ps.tile([P, 512], f32, tag="pqkt",
                                    name=f"pq_{hp}_{qc}")
                    for ep in range(EP):
                        nc.tensor.matmul(
                            pq[:],
                            wq_t[:, 2 * ep:2 * ep + 2, hp * P:(hp + 1) * P],
                            hqT_g[qc][:, 2 * ep:2 * ep + 2, :],
                            start=(ep == 0), stop=(ep == EP - 1), perf_mode=DR)
                    nc.scalar.activation(
                        QT_h[hp][:, qc * 512:(qc + 1) * 512],
                        pq[:], AF.Identity,
                        bias=bqc[:, hp:hp + 1], scale=1.0 / SW)

        for _pool in (warm1_ps, pv_ps, lns, lnp, pq_ps):
            _pool.release()

        # prefetch the projection weight + residual rows while attention runs
        nc.sync.dma_start(wp_t[:], wp_d.ap().rearrange("(o p) f -> p o f", p=P))
        nc.sync.dma_start(xq_s[:],
                          xq_d.ap().rearrange("(q p) c -> p q c", p=P))

        # ---------------- Phase 2: attention ------------------------------
        pr = tc.alloc_tile_pool(name="pr", bufs=3)
        rsn = tc.alloc_tile_pool(name="rsn", bufs=3)
        psc_ps = tc.alloc_tile_pool(name="psc", bufs=2, space="PSUM")
        pso_ps = tc.alloc_tile_pool(name="pso", bufs=2, space="PSUM")

        pending = [None]

        def emit_norm():
            # normalization of the PREVIOUS (hp, qc): raw row-sums (poA row
            # 64, poB row 32) -> SBUF, PE-broadcast into the GARBAGE
            # partition regions of the po banks themselves, wide-reciprocal
            # into SBUF, fp8 eviction fuses (po * SO/SV) * (1/rowsum).
            if pending[0] is None:
                return
            poA, poB, hp, qc = pending[0]
            pending[0] = None
            po_t = [poA, poB]
            c0 = rsn.tile([1, 512], bf16, tag="c0", name=f"c0_{hp}_{qc}")
            nc.vector.tensor_copy(out=c0[:], in_=poA[64:65, :])
            c1 = rsn.tile([1, 512], bf16, tag="c1", name=f"c1_{hp}_{qc}")
            nc.vector.tensor_copy(out=c1[:], in_=poB[32:33, :])
            nc.tensor.matmul(poB[0:64, :], ones1[:, 0:64], c0[:],
                             start=True, stop=True, skip_group_check=True)
            nc.tensor.matmul(poA[64:128, :], ones1[:, 0:64], c1[:],
                             start=True, stop=True, skip_group_check=True)
            pbs = rsn.tile([P, 512], bf16, tag="pbs", name=f"pbs_{hp}_{qc}")
            with nc.allow_low_precision(reason="1/rowsum as bf16 multiply "
                                        "operand"):
                nc.vector.reciprocal(pbs[0:64, :], poB[0:64, :])
                nc.vector.reciprocal(pbs[64:128, :], poA[64:128, :])
            for ab in range(2):
                lo = 64 * ab
                nc.vector.scalar_tensor_tensor(
                    outT_g[qc][lo:lo + 64, hp, :],
                    po_t[ab][lo:lo + 64, :], SO / SV,
                    pbs[lo:lo + 64, :], ALU.mult, ALU.mult)

        for hp in range(HP):
            for qc in range(2):
                q0 = qc * 512
                poA = pso_ps.tile([P, 512], f32, tag="poA", name=f"poA_{hp}_{qc}")
                poB = pso_ps.tile([P, 512], f32, tag="poB", name=f"poB_{hp}_{qc}")
                po_t = [poA, poB]
                npairs = 4 if qc == 0 else 8
                prb_h = {}

                def emit_scores(jp):
                    qsj = jp * P
                    qs = max(qsj, q0)
                    off = qs - q0
                    N = 512 - off
                    diag = (qs == qsj)
                    prb = pr.tile([P, 2, 2, 512], fp8, tag="prb", bufs=6,
                                  name=f"prb_{hp}_{qc}_{jp}")
                    prb_h[jp] = (prb, off, N)
                    for sl in range(2):
                        j = 2 * jp + sl
                        psc = psc_ps.tile([P, 2, 512], f32, tag="psc",
                                          name=f"psc_{hp}_{qc}_{j}")
                        for ab in range(2):
                            nc.tensor.matmul(
                                psc[:, ab, off:off + N],
                                KT_h[hp][64 * ab:64 * (ab + 1),
                                         j * P:(j + 1) * P],
                                QT_h[hp][64 * ab:64 * (ab + 1), qs:qs + N],
                                start=True, stop=not diag,
                                tile_position=(64 * ab, 0),
                                skip_group_check=True)
                        if diag:
                            nc.tensor.matmul(
                                psc[:, :, off:off + P],
                                eye[:], mask2[:, j % 2, :, :],
                                start=False, stop=True,
                                skip_group_check=True)
                        nc.scalar.activation(prb[:, sl, :, off:off + N],
                                             psc[:, :, off:off + N],
                                             AF.Exp, scale=0.125)

                # scores run one jp ahead of AV so the PE never waits a
                # full exp latency at the head-pair boundary
                emit_scores(0)
                for jp in range(npairs):
                    if jp + 1 < npairs:
                        emit_scores(jp + 1)
                    if jp == 0:
                        emit_norm()
                    prb, off, N = prb_h.pop(jp)
                    for ab in range(2):
                        nc.tensor.matmul(
                            po_t[ab][:, off:off + N],
                            V8[:, 2 * jp:2 * jp + 2, 2 * hp + ab, :],
                            prb[:, :, ab, off:off + N],
                            start=(jp == 0), stop=(jp == npairs - 1),
                            perf_mode=DR)
                pending[0] = (poA, poB, hp, qc)
        emit_norm()

        for _pool in (pso_ps, psc_ps, rsn, pr):
            _pool.release()
        w13.release()
        attnp.release()

        # -------- Phase 3: projection + residual + LN2 --------------------
        ffn = tc.alloc_tile_pool(name="ffn", bufs=1)
        w18_t = ffn.tile([P, 4, FF], fp8)
        nc.sync.dma_start(w18_t[:],
                          w1a_d.ap().rearrange("(o p) f -> p o f", p=P))
        w116_t = ffn.tile([P, 2, FF], bf16)
        nc.sync.dma_start(w116_t[:],
                          w1b_d.ap().rearrange("(o p) f -> p o f", p=P))
        w2_t = ffn.tile([P, MB, C], fp8)
        nc.sync.dma_start(w2_t[:],
                          w2_d.ap().rearrange("(m p) f -> p m f", p=P))
        uT_g = [ffn.tile([P, MB, 512], fp8, tag=f"uT{g}", name=f"uT{g}")
                for g in range(2)]

        lnp2 = tc.alloc_tile_pool(name="lnp2", bufs=4)
        lns2 = tc.alloc_tile_pool(name="lns2", bufs=4)
        ppr_ps = tc.alloc_tile_pool(name="ppr", bufs=2, space="PSUM")
        tr2_ps = tc.alloc_tile_pool(name="tr2_ps", bufs=2, space="PSUM")
        pu_ps = tc.alloc_tile_pool(name="pu", bufs=2, space="PSUM")

        # projection + residual: the PSUM descale and the residual add
        # fuse into one DVE scalar_tensor_tensor per half-block
        def proj_block(qb):
            for fo in range(2):
                pp = ppr_ps.tile([P, 384], f32, tag="ppr", name=f"pp_{qb}_{fo}")
                for fp in range(EP):
                    nc.tensor.matmul(
                        pp[:],
                        outT_g[qb // 4][:, 2 * fp:2 * fp + 2,
                                        (qb % 4) * P:(qb % 4 + 1) * P],
                        wp_t[:, 2 * fp:2 * fp + 2, fo * 384:(fo + 1) * 384],
                        start=(fp == 0), stop=(fp == EP - 1), perf_mode=DR)
                nc.vector.scalar_tensor_tensor(
                    x2s[:, qb, fo * 384:(fo + 1) * 384],
                    pp[:], 1.0 / (SO * SP_),
                    xq_s[:, qb, fo * 384:(fo + 1) * 384],
                    ALU.mult, ALU.add)

        # batched LN2 per group of 4 blocks, FFN1 for a group issued
        # right after its LN2 so the PE fills while the other group's
        # LN2 stats run on Act/DVE/GpSimd
        def ffn1_group(qc2):
            for mb in range(MB):
                pu = pu_ps.tile([P, 512], f32, tag="pu",
                                name=f"pu_{mb}_{qc2}")
                for dp in range(2):
                    nc.tensor.matmul(
                        pu[:],
                        w18_t[:, 2 * dp:2 * dp + 2, mb * P:(mb + 1) * P],
                        h2T8_g[qc2][:, 2 * dp:2 * dp + 2, :],
                        start=(dp == 0), stop=False, perf_mode=DR)
                for eo in range(2):
                    nc.tensor.matmul(
                        pu[:], w116_t[:, eo, mb * P:(mb + 1) * P],
                        h2T16_g[qc2][:, eo, :],
                        start=False, stop=(eo == 1))
                nc.scalar.activation(uT_g[qc2][:, mb, :], pu[:], AF.Relu,
                                     bias=b1c[:, mb:mb + 1], scale=SU / SW)

        def ln2_stats(g):
            s1g = lns2.tile([P, 4], f32, tag="l2_s1g", name=f"l2s1g{g}")
            nc.vector.tensor_reduce(s1g[:], x2s[:, 4 * g:4 * g + 4, :],
                                    mybir.AxisListType.X, ALU.add)
            s2g = lns2.tile([P, 4], f32, tag="l2_s2g", name=f"l2s2g{g}")
            for i in range(4):
                sq = lnp2.tile([P, C], bf16, tag="l2_sq", name=f"l2sq{g}_{i}")
                nc.scalar.activation(sq[:], x2s[:, 4 * g + i, :], AF.Square,
                                     accum_out=s2g[:, i:i + 1])
            mug = lns2.tile([P, 4], f32, tag="l2_mug", name=f"l2mug{g}")
            nc.gpsimd.tensor_scalar(mug[:], s1g[:], 1.0 / C, None, ALU.mult)
            mu2g = lns2.tile([P, 4], f32, tag="l2_mu2g", name=f"l2mu2g{g}")
            nc.gpsimd.tensor_tensor(mu2g[:], mug[:], mug[:], ALU.mult)
            veg = lns2.tile([P, 4], f32, tag="l2_veg", name=f"l2veg{g}")
            nc.gpsimd.tensor_scalar(veg[:], s2g[:], 1.0 / C, 1e-5,
                                    ALU.mult, ALU.add)
            nc.gpsimd.tensor_tensor(veg[:], veg[:], mu2g[:], ALU.subtract)
            sdg = lns2.tile([P, 4], f32, tag="l2_sdg", name=f"l2sdg{g}")
            nc.scalar.activation(sdg[:], veg[:], AF.Sqrt)
            rstdg = lns2.tile([P, 4], f32, tag="l2_rstdg", name=f"l2rstdg{g}")
            nc.vector.reciprocal(rstdg[:], sdg[:])
            nbg = lns2.tile([P, 4], f32, tag="l2_nbg", name=f"l2nbg{g}")
            nc.gpsimd.tensor_tensor(nbg[:], mug[:], rstdg[:], ALU.mult)
            nc.gpsimd.tensor_scalar(nbg[:], nbg[:], -1.0, None, ALU.mult)
            return rstdg, nbg

        def ln2_normalize(g, rstdg, nbg):
            for i in range(4):
                qb = 4 * g + i
                z = lnp2.tile([P, C], bf16, tag="l2_z", name=f"l2z{qb}")
                nc.vector.tensor_scalar(z[:], x2s[:, qb, :],
                                        rstdg[:, i:i + 1], nbg[:, i:i + 1],
                                        ALU.mult, ALU.add)
                tb = tr2_ps.tile([P, C], bf16, tag="l2_tb", name=f"l2tb{qb}")
                for eo in range(EO):
                    nc.tensor.matmul(tb[:, eo * P:(eo + 1) * P],
                                     z[:, eo * P:(eo + 1) * P], eye[:],
                                     is_transpose=True,
                                     skip_group_check=True)
                nc.scalar.activation(h2T8_g[g][:, :, i * P:(i + 1) * P],
                                     tb[:, 0:512].rearrange(
                                         "p (e q) -> p e q", e=4),
                                     AF.Identity)
                nc.vector.tensor_copy(
                    out=h2T16_g[g][:, :, i * P:(i + 1) * P],
                    in_=tb[:, 512:768].rearrange("p (e q) -> p e q", e=2))

        for qb in range(4):
            proj_block(qb)
        st0 = ln2_stats(0)
        for qb in range(4, NQ):
            proj_block(qb)
        ln2_normalize(0, *st0)
        st1 = ln2_stats(1)
        ffn1_group(0)
        ln2_normalize(1, *st1)
        ffn1_group(1)

        # -------- Phase 5: FFN2 -------------------------------------------
        oup = tc.alloc_tile_pool(name="oup", bufs=4)
        py_ps = tc.alloc_tile_pool(name="py", bufs=2, space="PSUM")

        for qb in range(NQ):
            ot = oup.tile([P, C], f32, tag="ot", name=f"ot_{qb}")
            for fo in range(2):
                py = py_ps.tile([P, 384], f32, tag="py", name=f"py_{qb}_{fo}")
                for mp in range(MP):
                    nc.tensor.matmul(
                        py[:],
                        uT_g[qb // 4][:, 2 * mp:2 * mp + 2,
                                      (qb % 4) * P:(qb % 4 + 1) * P],
                        w2_t[:, 2 * mp:2 * mp + 2, fo * 384:(fo + 1) * 384],
                        start=(mp == 0),
                        stop=(skip_b2 and mp == MP - 1), perf_mode=DR)
                if not skip_b2:
                    nc.tensor.matmul(py[:], ones1[:],
                                     b2r[:, fo * 384:(fo + 1) * 384],
                                     start=False, stop=True)
                nc.vector.scalar_tensor_tensor(
                    ot[:, fo * 384:(fo + 1) * 384], py[:], 1.0 / (SU * SW),
                    x2s[:, qb, fo * 384:(fo + 1) * 384],
                    ALU.mult, ALU.add)
            nc.sync.dma_start(out_d.ap()[qb * P:(qb + 1) * P, :], ot[:])

        for _pool in (py_ps, oup, pu_ps, tr2_ps, ppr_ps, lns2,
                      lnp2, ffn, xqp, late, pers):
            _pool.release()

    return nc


def prepare_in_maps(inputs):
    """Build the 8 per-core input maps from the full problem inputs."""
    x = np.asarray(inputs["x"], np.float32)
    wq = np.asarray(inputs["wq"], np.float32)
    wk = np.asarray(inputs["wk"], np.float32)
    wv = np.asarray(inputs["wv"], np.float32)
    w_proj = np.asarray(inputs["w_proj"], np.float32)
    b_proj = np.asarray(inputs["b_proj"], np.float32)
    w1 = np.asarray(inputs["w1"], np.float32)
    b1 = np.asarray(inputs["b1"], np.float32)
    w2 = np.asarray(inputs["w2"], np.float32)
    b2 = np.asarray(inputs["b2"], np.float32)
    g1 = np.asarray(inputs["ln1_g"], np.float32)
    be1 = np.asarray(inputs["ln1_b"], np.float32)
    g2 = np.asarray(inputs["ln2_g"], np.float32)
    be2 = np.asarray(inputs["ln2_b"], np.float32)

    bf = ml_dtypes.bfloat16
    f8 = ml_dtypes.float8_e4m3

    def to8(a):
        return np.clip(a, -224.0, 224.0).astype(f8)

    wq_r = wq.transpose(1, 0, 2).reshape(C, C)       # [c, h*d]
    wk_r = wk.transpose(1, 0, 2).reshape(C, C)
    wv_r = wv.transpose(1, 0, 2).reshape(C, C)
    wq8 = to8(SW * g1[:, None] * wq_r)
    wk8 = to8(SW * g1[:, None] * wk_r)
    wv8 = to8(SW * g1[:, None] * wv_r)
    bq = (be1 @ wq_r).reshape(HP, P).T.copy().astype(np.float32)   # [128, hp]
    bk = (be1 @ wk_r).reshape(HP, P).T.copy().astype(np.float32)
    bv_vec = (be1 @ wv_r).astype(np.float32)        # v bias, concat-head
    w1_s = SW * g2[:, None] * w1
    w1a8 = to8(w1_s[:512])
    w1b16 = w1_s[512:].astype(bf)
    b1f = (SU * (b1 + be2 @ w1)).reshape(MB, P).T.copy().astype(np.float32)
    wp8 = to8(SP_ * w_proj)
    w2_16 = to8(SW * w2)
    b2r = (SU * SW * b2).reshape(1, C).astype(bf)
    # bv and b_proj pass through attention/proj as constant rows; fold into xq
    xq_bias = (b_proj + bv_vec @ w_proj).astype(np.float32)

    ci = np.arange(P)[:, None]
    qi = np.arange(P)[None, :]
    tri_add = np.where(ci <= qi, 0.0, MASK_NEG).astype(np.float32)
    zer = np.zeros((P, P), np.float32)
    allm = np.full((P, P), MASK_NEG, np.float32)
    # mask2[c, jm, ab, q]: additive mask for diagonal key-block parity jm
    m_par = []
    for p in range(2):
        if p == 0:
            m0, m1 = tri_add, allm
        else:
            m0, m1 = zer, tri_add
        mk = np.stack([np.stack([m0, m0], 0), np.stack([m1, m1], 0)], 0)
        # mk[jm, ab, c, q] -> [c, jm, ab, q]
        m_par.append(np.ascontiguousarray(mk.transpose(2, 0, 1, 3)).astype(bf))

    eye16 = np.eye(P, dtype=np.float32).astype(bf)

    in_maps = []
    for core in range(8):
        b, p = core // 2, core % 2
        xf = np.ascontiguousarray(x[b]).astype(bf)
        xq = np.ascontiguousarray(
            x[b].reshape(NB, P, C)[p::2].reshape(TQ, C)) + xq_bias
        s0 = np.full((P, 1), 1.0 - p, np.float32)
        s1 = np.full((P, 1), float(p), np.float32)
        in_maps.append({
            "xq": xq, "xf": xf,
            "wq": wq8, "wk": wk8, "wv": wv8, "wp": wp8,
            "w1a": w1a8, "w1b": w1b16, "w2": w2_16,
            "bq": bq, "bk": bk, "b1": b1f, "b2": b2r,
            "mk": m_par[p], "eye": eye16, "s0": s0, "s1": s1,
        })
    return in_maps


def assemble_output(results):
    """Reassemble the 8 per-core [1024, 768] outputs into [4, 2048, 768]."""
    out = np.empty((4, T, C), np.float32)
    for core in range(8):
        b, p = core // 2, core % 2
        blocks = results[core]["out"].reshape(NQ, P, C)
        ov = out[b].reshape(NB, P, C)
        ov[p::2] = blocks
    return out


_CACHED_NC = None


def kernel(**inputs) -> np.ndarray:
    global _CACHED_NC
    from concourse.bass_utils import run_bass_kernel_spmd

    skip_b2 = bool(np.all(np.asarray(inputs["b2"]) == 0.0))
    if _CACHED_NC is None or _CACHED_NC[1] != skip_b2:
        nc = build_program(skip_b2=skip_b2)
        split_multiwait_instructions(nc)
        _CACHED_NC = (nc, skip_b2)
    in_maps = prepare_in_maps(inputs)
    res = run_bass_kernel_spmd(_CACHED_NC[0], in_maps, list(range(8)))
    return assemble_output(res.results)
